# revision 22
# baseline (speedup 1.0000x reference)
"""Exaone GQA flash-attention block on 8 Trainium2 NeuronCores.

Sharding: each pair of cores (2p, 2p+1) handles prefill sequence p (S=1024).
Within a pair, q-tokens are split by 256-blocks {0,3} / {1,2} so causal attention
work balances; K/V are computed per-core for the context each core needs
(zero-padded to 1024). No cross-core communication: every core produces the
final output rows for its own 512 q-tokens; the host concatenates.

All data layout work (transposes, bf16 casts, weight slot packing, cos/sin
broadcast tables, causal masks) happens on the HOST; the device runs a pure
matmul/softmax pipeline:
  V/K projections stream over pre-transposed bf16 hsT; K gets NeoX rope via a
      +-1 rotation matmul and two multiply-adds; V lands naturally [tok, ch]
      with an appended ones column so the PV matmul also produces the softmax
      denominator
  Q projection per head-slot from pre-packed weight slots, roped into a
      [128, 16 slots, 512] layout (row halves = kv-head parity so score
      matmuls row-pack the PE array)
  scoresT = kT^T @ qT per (kv, 256-q-chunk, key-block), exp on ACT,
      multiplicative triangle masks on the possible diagonal positions
  attn^T accumulated in PSUM [65, 4 heads, 128], normalized by the broadcast
      reciprocal of the ones-row, written straight into the out-proj lhsT
      layout; out = attn^T.T @ Wo in 512-wide chunks, interleaved into the
      attention instruction stream to fill PE bubbles.
"""
import sys
sys.path.insert(0, '/opt/trn_rl_repo')

from contextlib import ExitStack

import ml_dtypes
import numpy as np

import concourse.bass as bass
import concourse.mybir as mybir
import concourse.tile as tile
from concourse import bacc
from concourse.bass_utils import run_bass_kernel_spmd

F32 = mybir.dt.float32
BF16 = mybir.dt.bfloat16
AF = mybir.ActivationFunctionType
MUL = mybir.AluOpType.mult
ADD = mybir.AluOpType.add

B, S, D = 4, 1024, 2048
HQ, HKV, HD = 32, 8, 64
SCALE = HD ** -0.5
NQ = 512                      # q tokens per core
CSLOT2 = (4, 8)               # key-blocks processed per 256-q-chunk (uniform)
MASK_POS2 = ((0, 1, 2, 3), (4, 5, 6, 7))  # masked kb positions per 256-chunk


def build_nc():
    nc = bacc.Bacc("TRN2", target_bir_lowering=False, debug=False,
                   num_devices=8, num_swdge_queues=4)

    # all inputs pre-cast/pre-laid-out by the host
    hsc_d = nc.dram_tensor("hsct", [4, 128, 16, 256], BF16, kind="ExternalInput")
    hsq_d = nc.dram_tensor("hsqt", [2, 128, 16, 256], BF16, kind="ExternalInput")
    c4k_d = nc.dram_tensor("c4k", [128, S], BF16, kind="ExternalInput")
    s4k_d = nc.dram_tensor("s4k", [128, S], BF16, kind="ExternalInput")
    c4q_d = nc.dram_tensor("c4q", [128, NQ], BF16, kind="ExternalInput")
    s4q_d = nc.dram_tensor("s4q", [128, NQ], BF16, kind="ExternalInput")
    wq_d = nc.dram_tensor("wqs", [128, 16, 16, 128], BF16, kind="ExternalInput")
    wk_d = nc.dram_tensor("wks", [128, 4, 16, 128], BF16, kind="ExternalInput")
    wv_d = nc.dram_tensor("wvs", [128, 16, 512], BF16, kind="ExternalInput")
    wo_d = nc.dram_tensor("wos", [128, 4, 16, 512], BF16, kind="ExternalInput")
    rot_in = nc.dram_tensor("rot", [128, 128], BF16, kind="ExternalInput")
    masks_in = nc.dram_tensor("masks", [128, 2, 4, 256], BF16, kind="ExternalInput")
    out = nc.dram_tensor("out", [NQ, D], F32, kind="ExternalOutput")

    with tile.TileContext(nc) as tc:
        with ExitStack() as ctx:
            pool = lambda *a, **k: ctx.enter_context(tc.tile_pool(*a, **k))
            qT_p = pool(name="qT", bufs=1)
            kT_p = pool(name="kT", bufs=1)
            v_p = pool(name="vsb", bufs=1)
            attn_p = pool(name="attn", bufs=1)
            const_p = pool(name="const", bufs=1)
            exp_p = pool(name="exps", bufs=6)
            rope_p = pool(name="rope", bufs=2)
            norm_p = pool(name="norm", bufs=1)
            osb_p = pool(name="osb", bufs=2)

            qT = qT_p.tile([128, 16, NQ], BF16)
            kT = kT_p.tile([128, 4, S], BF16)
            v_sb = v_p.tile([128, 8, 8, 65], BF16)
            attn_sb = attn_p.tile([128, 16, NQ], BF16)

            # ---- constants (tiles only; DMAs issued after the critical
            #      startup transfers) ----
            rot_bf = const_p.tile([128, 128], BF16)
            c4q = const_p.tile([128, NQ], BF16)
            s4q = const_p.tile([128, NQ], BF16)
            masks_bf = const_p.tile([128, 2, 4, 256], BF16)
            c4k = const_p.tile([128, S], BF16)
            s4k = const_p.tile([128, S], BF16)

            nc.vector.memset(v_sb[:, :, :, 64], 1.0)

            def rope(psum, rps_pool, c4, s4, col0, n, dst, cp_eng=None):
                """psum [128, n] -> dst (bf16) with NeoX rope applied."""
                x_sb = rope_p.tile([128, n], BF16, tag="rsb")
                (cp_eng or nc.scalar.copy)(x_sb[:], psum[:])
                pr = rps_pool.tile([128, n], F32, tag="rps")
                nc.tensor.matmul(pr[:], rot_bf[:], x_sb[:], start=True, stop=True)
                t1 = rope_p.tile([128, n], BF16, tag="rt1")
                nc.vector.tensor_tensor(t1[:], pr[:], s4[:, col0:col0 + n], MUL)
                t2 = rope_p.tile([128, n], BF16, tag="rt2")
                nc.vector.tensor_tensor(t2[:], x_sb[:], c4[:, col0:col0 + n], MUL)
                nc.vector.tensor_tensor(dst, t1[:], t2[:], ADD)

            # ---- shared attention helpers ----
            def sc_exp(sc_ps, sl2, g, kb):
                a, par = g // 2, g % 2
                base = 64 * par
                sc = sc_ps.tile([128, 2, 4, 128], F32, tag="sc")
                for h in range(2):
                    nc.tensor.matmul(
                        sc[:, h],
                        kT[base:base + 64, a, kb * 128:(kb + 1) * 128],
                        qT[base:base + 64, 4 * a:4 * a + 4,
                           (2 * sl2 + h) * 128:(2 * sl2 + h + 1) * 128],
                        start=True, stop=True)
                ex = exp_p.tile([128, 2, 4, 128], BF16, tag="ex")
                nc.scalar.activation(ex[:], sc[:], AF.Exp, scale=SCALE)
                if kb in MASK_POS2[sl2]:
                    mi = MASK_POS2[sl2].index(kb)
                    mk = masks_bf[:, sl2, mi].rearrange(
                        "p (two x) -> p two x", two=2)[:, :, None, :]
                    nc.vector.tensor_tensor(
                        ex[:], ex[:], mk.to_broadcast((128, 2, 4, 128)), MUL)
                return ex

            def pv_kb(pvs, g, kb, ex, nkb):
                for h in range(2):
                    nc.tensor.matmul(
                        pvs[h][:], v_sb[:, kb, g, :], ex[:, h],
                        start=(kb == 0), stop=(kb == nkb - 1))

            def normalize(sl2, g, pvs):
                l_sb = norm_p.tile([1, 2, 4, 128], F32, tag="lsb")
                nc.vector.tensor_copy(l_sb[:, 0], pvs[0][64:65, :, :])
                nc.vector.tensor_copy(l_sb[:, 1], pvs[1][64:65, :, :])
                rc2 = norm_p.tile([1, 2, 4, 128], F32, tag="recip")
                nc.vector.reciprocal_approx_fast(
                    rc2.rearrange("p a b q -> p (a b q)"),
                    l_sb.rearrange("p a b q -> p (a b q)"))
                rb2 = norm_p.tile([64, 2, 4, 128], F32, tag="rb")
                nc.gpsimd.partition_broadcast(rb2[:], rc2[:])
                for h in range(2):
                    sl = 2 * sl2 + h
                    pv = pvs[h]
                    pv_pair = pv[0:64].rearrange(
                        "p (i two) q -> p two i q", two=2)
                    rb_pair = rb2[:, h].rearrange(
                        "p (i two) q -> p two i q", two=2)
                    for par_o in range(2):
                        nc.vector.tensor_tensor(
                            attn_sb[64 * par_o:64 * par_o + 64,
                                    2 * g:2 * g + 2,
                                    sl * 128:(sl + 1) * 128],
                            pv_pair[:, par_o], rb_pair[:, par_o], MUL)

            # ---- phase A1: V + K projections ----
            with ExitStack() as ictx:
                ipool = lambda *a, **k: ictx.enter_context(tc.tile_pool(*a, **k))
                hsc_p = ipool(name="hsc", bufs=1)
                wk_p = ipool(name="wks", bufs=1)
                wv_p2 = ipool(name="wvs", bufs=1)
                proj_ps = ipool(name="proj_ps", bufs=3, space="PSUM")
                rot_ps = ipool(name="rot_ps", bufs=2, space="PSUM")

                # sync queue: wv half + hsc 2,3 + wk; scalar: hsc 0,1 then
                # the Q path; gpsimd: wv half, then k-rope tables + masks
                # startup-critical bytes first, in consumption order and
                # split so the first matmuls start after ~1.5MB; gpsimd
                # (slow SWDGE) carries nothing early
                wv_bf = wv_p2.tile([128, 16, 512], BF16)
                hs_ctxT = hsc_p.tile([128, 4, 16, 256], BF16)
                nc.sync.dma_start(wv_bf[:, 0:8], wv_d[:, 0:8])
                nc.scalar.dma_start(hs_ctxT[:, 0, 0:8], hsc_d[0, :, 0:8])
                nc.sync.dma_start(wv_bf[:, 8:16], wv_d[:, 8:16])
                nc.scalar.dma_start(hs_ctxT[:, 0, 8:16], hsc_d[0, :, 8:16])
                nc.scalar.dma_start(hs_ctxT[:, 1], hsc_d[1])
                for c in (2, 3):
                    nc.sync.dma_start(hs_ctxT[:, c], hsc_d[c])
                wk_bf = wk_p.tile([128, 4, 16, 128], BF16)
                nc.sync.dma_start(wk_bf[:], wk_d[:])
                nc.scalar.dma_start(rot_bf[:], rot_in[:])
                nc.scalar.dma_start(c4k[:], c4k_d[:])
                nc.scalar.dma_start(s4k[:], s4k_d[:])
                nc.scalar.dma_start(c4q[:], c4q_d[:])
                nc.scalar.dma_start(s4q[:], s4q_d[:])
                nc.scalar.dma_start(masks_bf[:], masks_in[:])

                def v_tile(tt):
                    pv32 = proj_ps.tile([128, 512], F32, tag="proj")
                    for kt in range(16):
                        nc.tensor.matmul(
                            pv32[:],
                            hs_ctxT[:, tt // 2, kt,
                                    (tt % 2) * 128:(tt % 2) * 128 + 128],
                            wv_bf[:, kt, :], start=(kt == 0), stop=(kt == 15))
                    nc.vector.tensor_copy(
                        v_sb[:, tt, :, 0:64],
                        pv32.rearrange("p (g c) -> p g c", g=8))

                def k_chain(p, ch):
                    pk = proj_ps.tile([128, 512], F32, tag="proj")
                    for kt in range(16):
                        nc.tensor.matmul(
                            pk[:], wk_bf[:, p, kt, :],
                            hs_ctxT[:, 2 * ch:2 * ch + 2, kt, :],
                            start=(kt == 0), stop=(kt == 15))
                    rope(pk, rot_ps, c4k, s4k, 512 * ch, 512,
                         kT[:, p, 512 * ch:512 * (ch + 1)])

                v_tile(0)
                v_tile(1)
                v_tile(2)
                v_tile(3)
                k_chain(0, 0)
                k_chain(1, 0)
                k_chain(2, 0)
                k_chain(3, 0)
                v_tile(4)
                v_tile(5)
                v_tile(6)
                v_tile(7)
                k_chain(0, 1)
                k_chain(1, 1)
                k_chain(2, 1)
                k_chain(3, 1)

            # ---- phase A2: Q projection with the sl2=0 attention groups
            #      interleaved (Q matmuls keep the PE dense and warm while
            #      the exps run on the otherwise-idle ACT engine) ----
            wo_p = ctx.enter_context(tc.tile_pool(name="wo", bufs=4))
            wo_bfs = []
            for oc in range(4):
                wo_bf = wo_p.tile([128, 16, 512], BF16, tag="wobf")
                nc.sync.dma_start(wo_bf[:], wo_d[:, oc])
                wo_bfs.append(wo_bf)

            with ExitStack() as a2ctx:
                apool = lambda *a, **k: a2ctx.enter_context(tc.tile_pool(*a, **k))
                hsq_p = apool(name="hsq", bufs=1)
                wq_p = apool(name="wqs", bufs=3)
                proj2_ps = apool(name="proj2_ps", bufs=1, space="PSUM")
                rot2_ps = apool(name="rot2_ps", bufs=1, space="PSUM")
                scA_ps = apool(name="scA_ps", bufs=2, space="PSUM")
                pvA_ps = apool(name="pvA_ps", bufs=2, space="PSUM")

                hs_qT = hsq_p.tile([128, 2, 16, 256], BF16)
                for c in range(2):
                    nc.sync.dma_start(hs_qT[:, c], hsq_d[c])

                wq_tiles = {}

                def q_slot(s):
                    if s % 2 == 0:
                        wqa = wq_p.tile([128, 2, 16, 128], BF16, tag="wqa")
                        nc.scalar.dma_start(wqa[:], wq_d[:, s:s + 2])
                        wq_tiles[s // 2] = wqa
                    wqa = wq_tiles[s // 2]
                    pq = proj2_ps.tile([128, 512], F32, tag="proj")
                    for kt in range(16):
                        nc.tensor.matmul(
                            pq[:], wqa[:, s % 2, kt, :], hs_qT[:, :, kt, :],
                            start=(kt == 0), stop=(kt == 15))
                    cp = nc.scalar.copy if s % 2 == 0 else nc.vector.tensor_copy
                    rope(pq, rot2_ps, c4q, s4q, 0, NQ, qT[:, s, :], cp_eng=cp)

                def attA_group(g):
                    nkb = CSLOT2[0]
                    pvs = [pvA_ps.tile([65, 4, 128], F32, tag="pv",
                                       name=f"pv{h}") for h in range(2)]
                    exs = {}
                    for kb in range(nkb):
                        exs[kb] = sc_exp(scA_ps, 0, g, kb)
                        if kb > 0:
                            pv_kb(pvs, g, kb - 1, exs[kb - 1], nkb)
                    pv_kb(pvs, g, nkb - 1, exs[nkb - 1], nkb)
                    normalize(0, g, pvs)

                q_slot(0)
                q_slot(1)
                q_slot(2)
                q_slot(3)
                attA_group(0)
                q_slot(4)
                q_slot(5)
                attA_group(1)
                q_slot(6)
                q_slot(7)
                attA_group(2)
                q_slot(8)
                q_slot(9)
                attA_group(3)
                q_slot(10)
                q_slot(11)
                attA_group(4)
                q_slot(12)
                q_slot(13)
                attA_group(5)
                q_slot(14)
                q_slot(15)

            # ---- phase A2b: the last two sl2=0 groups have no Q filler
            #      left; run them as an interleaved pair (disjoint PE row
            #      groups) so each one's exp hides behind the other's MMs ----
            with ExitStack() as a2b:
                bpool2 = lambda *a, **k: a2b.enter_context(tc.tile_pool(*a, **k))
                scT_ps = bpool2(name="scT_ps", bufs=2, space="PSUM")
                pvT_ps = bpool2(name="pvT_ps", bufs=4, space="PSUM")
                nkb = CSLOT2[0]
                pvs6 = [pvT_ps.tile([65, 4, 128], F32, tag="pv",
                                    name=f"pv6{h}") for h in range(2)]
                pvs7 = [pvT_ps.tile([65, 4, 128], F32, tag="pv",
                                    name=f"pv7{h}") for h in range(2)]
                exs = {}
                for kb in range(nkb):
                    exs[(6, kb)] = sc_exp(scT_ps, 0, 6, kb)
                    exs[(7, kb)] = sc_exp(scT_ps, 0, 7, kb)
                    if kb > 0:
                        pv_kb(pvs6, 6, kb - 1, exs[(6, kb - 1)], nkb)
                        pv_kb(pvs7, 7, kb - 1, exs[(7, kb - 1)], nkb)
                pv_kb(pvs6, 6, nkb - 1, exs[(6, nkb - 1)], nkb)
                pv_kb(pvs7, 7, nkb - 1, exs[(7, nkb - 1)], nkb)
                normalize(0, 6, pvs6)
                normalize(0, 7, pvs7)

            # ---- phase B: sl2=1 attention with the tt=0/1 out-projection
            #      matmuls spread through each group ----
            with ExitStack() as bctx:
                bpool = lambda *a, **k: bctx.enter_context(tc.tile_pool(*a, **k))
                scB_ps = bpool(name="scB_ps", bufs=2, space="PSUM")
                pvB_ps = bpool(name="pvB_ps", bufs=3, space="PSUM")
                po_ps = bpool(name="po_ps", bufs=1, space="PSUM")

                ochunks = [(tt, oc) for oc in range(4) for tt in range(2)]

                def o_finish(tt, oc, po):
                    o_sb = osb_p.tile([128, 512], F32, tag="osb")
                    nc.vector.tensor_copy(o_sb[:], po[:])
                    nc.sync.dma_start(
                        out[tt * 128:(tt + 1) * 128, 512 * oc:512 * (oc + 1)],
                        o_sb[:])

                nkb = CSLOT2[1]
                for g in range(8):
                    tt, oc = ochunks[g]
                    po = po_ps.tile([128, 512], F32, tag="po")
                    wo_bf = wo_bfs[oc]
                    pvs = [pvB_ps.tile([65, 4, 128], F32, tag="pv",
                                       name=f"pv{h}") for h in range(2)]
                    exs = {}
                    for kb in range(nkb):
                        exs[kb] = sc_exp(scB_ps, 1, g, kb)
                        for cht in (2 * kb, 2 * kb + 1):
                            nc.tensor.matmul(
                                po[:], attn_sb[:, cht, tt * 128:(tt + 1) * 128],
                                wo_bf[:, cht, :],
                                start=(cht == 0), stop=(cht == 15))
                        if kb > 0:
                            pv_kb(pvs, g, kb - 1, exs[kb - 1], nkb)
                    pv_kb(pvs, g, nkb - 1, exs[nkb - 1], nkb)
                    normalize(1, g, pvs)
                    o_finish(tt, oc, po)

                # remaining out-projection chunks (q tokens 256..511)
                for oc in range(4):
                    for tt in range(2, 4):
                        po = po_ps.tile([128, 512], F32, tag="po")
                        wo_bf = wo_bfs[oc]
                        for cht in range(16):
                            nc.tensor.matmul(
                                po[:], attn_sb[:, cht,
                                               tt * 128:(tt + 1) * 128],
                                wo_bf[:, cht, :],
                                start=(cht == 0), stop=(cht == 15))
                        o_finish(tt, oc, po)

    nc.finalize()
    return nc


def _core_rows(c):
    p, which = c // 2, c % 2
    if which == 0:
        rel = np.r_[np.arange(256), np.arange(768, 1024)]
        ctx = 1024
    else:
        rel = np.arange(256, 768)
        ctx = 768
    return p, rel, ctx


def _rot_host():
    rot = np.zeros((128, 128), np.float32)
    for o in (0, 64):
        for d in range(32):
            rot[o + 32 + d, o + d] = -1.0
            rot[o + d, o + 32 + d] = 1.0
    return rot.astype(ml_dtypes.bfloat16)


def _pack_hsT(hs):
    """[T, 2048] f32 -> [T//256, 128, 16, 256] bf16 (ki, ko, t)."""
    T = hs.shape[0]
    ht = hs.T.astype(ml_dtypes.bfloat16)            # [2048, T]
    ht = ht.reshape(16, 128, T // 256, 256)         # ko, ki, c, t
    return np.ascontiguousarray(ht.transpose(2, 1, 0, 3))


def _cs_table(x):
    """cos/sin [T, 32] f32 -> [128, T] bf16 broadcast over 4 row groups."""
    t = np.tile(x.T.astype(ml_dtypes.bfloat16), (4, 1))
    return np.ascontiguousarray(t)


_NC_CACHE = {}
_LAST_INMAPS = None


def kernel(hidden_states, cos, sin, Wq, Wk, Wv, Wo):
    hidden_states = np.ascontiguousarray(hidden_states, dtype=np.float32)
    cos = np.ascontiguousarray(cos, dtype=np.float32)
    sin = np.ascontiguousarray(sin, dtype=np.float32)
    Wq = np.ascontiguousarray(Wq, dtype=np.float32)
    Wk = np.ascontiguousarray(Wk, dtype=np.float32)
    Wv = np.ascontiguousarray(Wv, dtype=np.float32)
    Wo = np.ascontiguousarray(Wo, dtype=np.float32)

    if "nc" not in _NC_CACHE:
        _NC_CACHE["nc"] = build_nc()
    nc = _NC_CACHE["nc"]

    # weight packs (shared across cores)
    wq_s = np.ascontiguousarray(
        Wq.reshape(16, 128, 4, 2, 4, 64)            # ko ki a r i x
        .transpose(1, 2, 4, 0, 3, 5)                # ki a i ko r x
        .reshape(128, 16, 16, 128).astype(ml_dtypes.bfloat16))
    wk_s = np.ascontiguousarray(
        Wk.reshape(16, 128, 4, 128).transpose(1, 2, 0, 3)
        .astype(ml_dtypes.bfloat16))
    wv_s = np.ascontiguousarray(
        Wv.reshape(16, 128, 512).transpose(1, 0, 2).astype(ml_dtypes.bfloat16))
    wo_s = np.ascontiguousarray(
        Wo.reshape(16, 128, 4, 512).transpose(1, 2, 0, 3)
        .astype(ml_dtypes.bfloat16))
    rot = _rot_host()

    in_maps = []
    for c in range(8):
        p, rel, ctx = _core_rows(c)
        rows = p * S + rel
        hs_ctx = np.zeros((S, D), np.float32)
        hs_ctx[:ctx] = hidden_states[p * S:p * S + ctx]
        masks = np.ones((128, 2, 4, 256), np.float32)
        for sl2 in range(2):
            qabs = rel[sl2 * 256:(sl2 + 1) * 256]
            for mi, pos in enumerate(MASK_POS2[sl2]):
                kabs = pos * 128 + np.arange(128)
                masks[:, sl2, mi, :] = (qabs[None, :] >= kabs[:, None])
        in_maps.append(dict(
            hsct=_pack_hsT(hs_ctx),
            hsqt=_pack_hsT(np.ascontiguousarray(hidden_states[rows])),
            c4k=_cs_table(cos[p * S:(p + 1) * S]),
            s4k=_cs_table(sin[p * S:(p + 1) * S]),
            c4q=_cs_table(cos[p * S + rel]),
            s4q=_cs_table(sin[p * S + rel]),
            wqs=wq_s, wks=wk_s, wvs=wv_s, wos=wo_s,
            rot=rot, masks=masks.astype(ml_dtypes.bfloat16),
        ))

    global _LAST_INMAPS
    _LAST_INMAPS = in_maps

    last_err = None
    for _attempt in range(2):
        try:
            res = run_bass_kernel_spmd(nc, in_maps, core_ids=list(range(8)))
            break
        except Exception as e:  # one retry: device occasionally needs a reset
            last_err = e
    else:
        raise last_err

    outp = np.zeros((B * S, D), np.float32)
    for c in range(8):
        p, rel, ctx = _core_rows(c)
        outp[p * S + rel] = res.results[c]["out"]
    return outp


# revision 23
# speedup vs baseline: 1.0369x; 1.0369x over previous
"""Exaone GQA flash-attention block on 8 Trainium2 NeuronCores.

Sharding: each pair of cores (2p, 2p+1) handles prefill sequence p (S=1024).
Within a pair, q-tokens are split by 256-blocks {0,3} / {1,2} so causal attention
work balances; K/V are computed per-core for the context each core needs
(zero-padded to 1024). No cross-core communication: every core produces the
final output rows for its own 512 q-tokens; the host concatenates.

All data layout work (transposes, bf16 casts, weight slot packing, cos/sin
broadcast tables, causal masks) happens on the HOST; the device runs a pure
matmul/softmax pipeline:
  V/K projections stream over pre-transposed bf16 hsT; K gets NeoX rope via a
      +-1 rotation matmul and two multiply-adds; V lands naturally [tok, ch]
      with an appended ones column so the PV matmul also produces the softmax
      denominator
  Q projection per head-slot from pre-packed weight slots, roped into a
      [128, 16 slots, 512] layout (row halves = kv-head parity so score
      matmuls row-pack the PE array)
  scoresT = kT^T @ qT per (kv, 256-q-chunk, key-block), exp on ACT,
      multiplicative triangle masks on the possible diagonal positions
  attn^T accumulated in PSUM [65, 4 heads, 128], normalized by the broadcast
      reciprocal of the ones-row, written straight into the out-proj lhsT
      layout; out = attn^T.T @ Wo in 512-wide chunks, interleaved into the
      attention instruction stream to fill PE bubbles.
"""
import sys
sys.path.insert(0, '/opt/trn_rl_repo')

from contextlib import ExitStack

import ml_dtypes
import numpy as np

import concourse.bass as bass
import concourse.mybir as mybir
import concourse.tile as tile
from concourse import bacc
from concourse.bass_utils import run_bass_kernel_spmd

F32 = mybir.dt.float32
BF16 = mybir.dt.bfloat16
AF = mybir.ActivationFunctionType
MUL = mybir.AluOpType.mult
ADD = mybir.AluOpType.add

B, S, D = 4, 1024, 2048
HQ, HKV, HD = 32, 8, 64
SCALE = HD ** -0.5
NQ = 512                      # q tokens per core
CSLOT2 = (4, 8)               # key-blocks processed per 256-q-chunk (uniform)
MASK_POS2 = ((0, 1, 2, 3), (4, 5, 6, 7))  # masked kb positions per 256-chunk


def build_nc():
    nc = bacc.Bacc("TRN2", target_bir_lowering=False, debug=False,
                   num_devices=8, num_swdge_queues=4)

    # all inputs pre-cast/pre-laid-out by the host
    hsc_d = nc.dram_tensor("hsct", [4, 128, 16, 256], BF16, kind="ExternalInput")
    hsq_d = nc.dram_tensor("hsqt", [2, 128, 16, 256], BF16, kind="ExternalInput")
    c4k_d = nc.dram_tensor("c4k", [128, S], BF16, kind="ExternalInput")
    s4k_d = nc.dram_tensor("s4k", [128, S], BF16, kind="ExternalInput")
    c4q_d = nc.dram_tensor("c4q", [128, NQ], BF16, kind="ExternalInput")
    s4q_d = nc.dram_tensor("s4q", [128, NQ], BF16, kind="ExternalInput")
    wq_d = nc.dram_tensor("wqs", [128, 16, 16, 128], BF16, kind="ExternalInput")
    wk_d = nc.dram_tensor("wks", [128, 4, 16, 128], BF16, kind="ExternalInput")
    wv_d = nc.dram_tensor("wvs", [128, 16, 512], BF16, kind="ExternalInput")
    wo_d = nc.dram_tensor("wos", [128, 4, 16, 512], BF16, kind="ExternalInput")
    rot_in = nc.dram_tensor("rot", [128, 128], BF16, kind="ExternalInput")
    masks_in = nc.dram_tensor("masks", [128, 2, 4, 256], BF16, kind="ExternalInput")
    out = nc.dram_tensor("out", [NQ, D], F32, kind="ExternalOutput")

    with tile.TileContext(nc) as tc:
        with ExitStack() as ctx:
            pool = lambda *a, **k: ctx.enter_context(tc.tile_pool(*a, **k))
            qT_p = pool(name="qT", bufs=1)
            kT_p = pool(name="kT", bufs=1)
            v_p = pool(name="vsb", bufs=1)
            attn_p = pool(name="attn", bufs=1)
            const_p = pool(name="const", bufs=1)
            exp_p = pool(name="exps", bufs=6)
            rope_p = pool(name="rope", bufs=2)
            norm_p = pool(name="norm", bufs=1)
            osb_p = pool(name="osb", bufs=2)

            qT = qT_p.tile([128, 16, NQ], BF16)
            kT = kT_p.tile([128, 4, S], BF16)
            v_sb = v_p.tile([128, 8, 8, 65], BF16)
            attn_sb = attn_p.tile([128, 16, NQ], BF16)

            # ---- constants (tiles only; DMAs issued after the critical
            #      startup transfers) ----
            rot_bf = const_p.tile([128, 128], BF16)
            c4q = const_p.tile([128, NQ], BF16)
            s4q = const_p.tile([128, NQ], BF16)
            masks_bf = const_p.tile([128, 2, 4, 256], BF16)
            c4k = const_p.tile([128, S], BF16)
            s4k = const_p.tile([128, S], BF16)

            nc.vector.memset(v_sb[:, :, :, 64], 1.0)

            def rope(psum, rps_pool, c4, s4, col0, n, dst, cp_eng=None):
                """psum [128, n] -> dst (bf16) with NeoX rope applied."""
                x_sb = rope_p.tile([128, n], BF16, tag="rsb")
                (cp_eng or nc.scalar.copy)(x_sb[:], psum[:])
                pr = rps_pool.tile([128, n], F32, tag="rps")
                nc.tensor.matmul(pr[:], rot_bf[:], x_sb[:], start=True, stop=True)
                t1 = rope_p.tile([128, n], BF16, tag="rt1")
                nc.vector.tensor_tensor(t1[:], pr[:], s4[:, col0:col0 + n], MUL)
                t2 = rope_p.tile([128, n], BF16, tag="rt2")
                nc.vector.tensor_tensor(t2[:], x_sb[:], c4[:, col0:col0 + n], MUL)
                nc.vector.tensor_tensor(dst, t1[:], t2[:], ADD)

            # ---- shared attention helpers ----
            def sc_exp(sc_ps, sl2, g, kb):
                a, par = g // 2, g % 2
                base = 64 * par
                sc = sc_ps.tile([128, 2, 4, 128], F32, tag="sc")
                for h in range(2):
                    nc.tensor.matmul(
                        sc[:, h],
                        kT[base:base + 64, a, kb * 128:(kb + 1) * 128],
                        qT[base:base + 64, 4 * a:4 * a + 4,
                           (2 * sl2 + h) * 128:(2 * sl2 + h + 1) * 128],
                        start=True, stop=True)
                ex = exp_p.tile([128, 2, 4, 128], BF16, tag="ex")
                nc.scalar.activation(ex[:], sc[:], AF.Exp, scale=SCALE)
                if kb in MASK_POS2[sl2]:
                    mi = MASK_POS2[sl2].index(kb)
                    mk = masks_bf[:, sl2, mi].rearrange(
                        "p (two x) -> p two x", two=2)[:, :, None, :]
                    nc.vector.tensor_tensor(
                        ex[:], ex[:], mk.to_broadcast((128, 2, 4, 128)), MUL)
                return ex

            def pv_kb(pvs, g, kb, ex, nkb):
                for h in range(2):
                    nc.tensor.matmul(
                        pvs[h][:], v_sb[:, kb, g, :], ex[:, h],
                        start=(kb == 0), stop=(kb == nkb - 1))

            def normalize(sl2, g, pvs):
                l_sb = norm_p.tile([1, 2, 4, 128], F32, tag="lsb")
                nc.vector.tensor_copy(l_sb[:, 0], pvs[0][64:65, :, :])
                nc.vector.tensor_copy(l_sb[:, 1], pvs[1][64:65, :, :])
                rc2 = norm_p.tile([1, 2, 4, 128], F32, tag="recip")
                nc.vector.reciprocal_approx_fast(
                    rc2.rearrange("p a b q -> p (a b q)"),
                    l_sb.rearrange("p a b q -> p (a b q)"))
                rb2 = norm_p.tile([64, 2, 4, 128], F32, tag="rb")
                nc.gpsimd.partition_broadcast(rb2[:], rc2[:])
                for h in range(2):
                    sl = 2 * sl2 + h
                    pv = pvs[h]
                    pv_pair = pv[0:64].rearrange(
                        "p (i two) q -> p two i q", two=2)
                    rb_pair = rb2[:, h].rearrange(
                        "p (i two) q -> p two i q", two=2)
                    for par_o in range(2):
                        nc.vector.tensor_tensor(
                            attn_sb[64 * par_o:64 * par_o + 64,
                                    2 * g:2 * g + 2,
                                    sl * 128:(sl + 1) * 128],
                            pv_pair[:, par_o], rb_pair[:, par_o], MUL)

            # ---- phase A1: V + K projections ----
            with ExitStack() as ictx:
                ipool = lambda *a, **k: ictx.enter_context(tc.tile_pool(*a, **k))
                hsc_p = ipool(name="hsc", bufs=1)
                wk_p = ipool(name="wks", bufs=1)
                wv_p2 = ipool(name="wvs", bufs=1)
                proj_ps = ipool(name="proj_ps", bufs=3, space="PSUM")
                rot_ps = ipool(name="rot_ps", bufs=2, space="PSUM")

                # sync queue: wv half + hsc 2,3 + wk; scalar: hsc 0,1 then
                # the Q path; gpsimd: wv half, then k-rope tables + masks
                # startup-critical bytes first, in consumption order and
                # split so the first matmuls start after ~1.5MB; gpsimd
                # (slow SWDGE) carries nothing early
                wv_bf = wv_p2.tile([128, 16, 512], BF16)
                hs_ctxT = hsc_p.tile([128, 4, 16, 256], BF16)
                nc.sync.dma_start(wv_bf[:, 0:8], wv_d[:, 0:8])
                nc.scalar.dma_start(hs_ctxT[:, 0, 0:8], hsc_d[0, :, 0:8])
                nc.sync.dma_start(wv_bf[:, 8:16], wv_d[:, 8:16])
                nc.scalar.dma_start(hs_ctxT[:, 0, 8:16], hsc_d[0, :, 8:16])
                nc.scalar.dma_start(hs_ctxT[:, 1], hsc_d[1])
                wk_bf = wk_p.tile([128, 4, 16, 128], BF16)
                nc.scalar.dma_start(wk_bf[:], wk_d[:])
                for c in (2, 3):
                    nc.sync.dma_start(hs_ctxT[:, c], hsc_d[c])
                nc.scalar.dma_start(rot_bf[:], rot_in[:])
                nc.scalar.dma_start(c4k[:], c4k_d[:])
                nc.scalar.dma_start(s4k[:], s4k_d[:])
                nc.scalar.dma_start(c4q[:], c4q_d[:])
                nc.scalar.dma_start(s4q[:], s4q_d[:])
                nc.scalar.dma_start(masks_bf[:], masks_in[:])

                def v_tile(tt):
                    pv32 = proj_ps.tile([128, 512], F32, tag="proj")
                    for kt in range(16):
                        nc.tensor.matmul(
                            pv32[:],
                            hs_ctxT[:, tt // 2, kt,
                                    (tt % 2) * 128:(tt % 2) * 128 + 128],
                            wv_bf[:, kt, :], start=(kt == 0), stop=(kt == 15))
                    nc.vector.tensor_copy(
                        v_sb[:, tt, :, 0:64],
                        pv32.rearrange("p (g c) -> p g c", g=8))

                def k_chain(p, ch):
                    pk = proj_ps.tile([128, 512], F32, tag="proj")
                    for kt in range(16):
                        nc.tensor.matmul(
                            pk[:], wk_bf[:, p, kt, :],
                            hs_ctxT[:, 2 * ch:2 * ch + 2, kt, :],
                            start=(kt == 0), stop=(kt == 15))
                    rope(pk, rot_ps, c4k, s4k, 512 * ch, 512,
                         kT[:, p, 512 * ch:512 * (ch + 1)])

                v_tile(0)
                v_tile(1)
                v_tile(2)
                v_tile(3)
                k_chain(0, 0)
                k_chain(1, 0)
                k_chain(2, 0)
                k_chain(3, 0)
                v_tile(4)
                v_tile(5)
                v_tile(6)
                v_tile(7)
                k_chain(0, 1)
                k_chain(1, 1)
                k_chain(2, 1)
                k_chain(3, 1)

            # ---- phase A2: Q projection with the sl2=0 attention groups
            #      interleaved (Q matmuls keep the PE dense and warm while
            #      the exps run on the otherwise-idle ACT engine) ----
            wo_p = ctx.enter_context(tc.tile_pool(name="wo", bufs=4))
            wo_bfs = []
            for oc in range(4):
                wo_bf = wo_p.tile([128, 16, 512], BF16, tag="wobf")
                nc.sync.dma_start(wo_bf[:], wo_d[:, oc])
                wo_bfs.append(wo_bf)

            with ExitStack() as a2ctx:
                apool = lambda *a, **k: a2ctx.enter_context(tc.tile_pool(*a, **k))
                hsq_p = apool(name="hsq", bufs=1)
                wq_p = apool(name="wqs", bufs=3)
                proj2_ps = apool(name="proj2_ps", bufs=1, space="PSUM")
                rot2_ps = apool(name="rot2_ps", bufs=1, space="PSUM")
                scA_ps = apool(name="scA_ps", bufs=2, space="PSUM")
                pvA_ps = apool(name="pvA_ps", bufs=2, space="PSUM")

                hs_qT = hsq_p.tile([128, 2, 16, 256], BF16)
                for c in range(2):
                    nc.sync.dma_start(hs_qT[:, c], hsq_d[c])

                wq_tiles = {}

                def q_slot(s):
                    if s % 2 == 0:
                        wqa = wq_p.tile([128, 2, 16, 128], BF16, tag="wqa")
                        nc.scalar.dma_start(wqa[:], wq_d[:, s:s + 2])
                        wq_tiles[s // 2] = wqa
                    wqa = wq_tiles[s // 2]
                    pq = proj2_ps.tile([128, 512], F32, tag="proj")
                    for kt in range(16):
                        nc.tensor.matmul(
                            pq[:], wqa[:, s % 2, kt, :], hs_qT[:, :, kt, :],
                            start=(kt == 0), stop=(kt == 15))
                    cp = nc.scalar.copy if s % 2 == 0 else nc.vector.tensor_copy
                    rope(pq, rot2_ps, c4q, s4q, 0, NQ, qT[:, s, :], cp_eng=cp)

                def attA_group(g):
                    nkb = CSLOT2[0]
                    pvs = [pvA_ps.tile([65, 4, 128], F32, tag="pv",
                                       name=f"pv{h}") for h in range(2)]
                    exs = {}
                    for kb in range(nkb):
                        exs[kb] = sc_exp(scA_ps, 0, g, kb)
                        if kb > 0:
                            pv_kb(pvs, g, kb - 1, exs[kb - 1], nkb)
                    pv_kb(pvs, g, nkb - 1, exs[nkb - 1], nkb)
                    normalize(0, g, pvs)

                q_slot(0)
                q_slot(1)
                q_slot(2)
                q_slot(3)
                attA_group(0)
                q_slot(4)
                q_slot(5)
                attA_group(1)
                q_slot(6)
                q_slot(7)
                attA_group(2)
                q_slot(8)
                q_slot(9)
                attA_group(3)
                q_slot(10)
                q_slot(11)
                attA_group(4)
                q_slot(12)
                q_slot(13)
                attA_group(5)
                q_slot(14)
                q_slot(15)

            # ---- phase A2b: the last two sl2=0 groups have no Q filler
            #      left; run them as an interleaved pair (disjoint PE row
            #      groups) so each one's exp hides behind the other's MMs ----
            with ExitStack() as a2b:
                bpool2 = lambda *a, **k: a2b.enter_context(tc.tile_pool(*a, **k))
                scT_ps = bpool2(name="scT_ps", bufs=2, space="PSUM")
                pvT_ps = bpool2(name="pvT_ps", bufs=4, space="PSUM")
                nkb = CSLOT2[0]
                pvs6 = [pvT_ps.tile([65, 4, 128], F32, tag="pv",
                                    name=f"pv6{h}") for h in range(2)]
                pvs7 = [pvT_ps.tile([65, 4, 128], F32, tag="pv",
                                    name=f"pv7{h}") for h in range(2)]
                exs = {}
                for kb in range(nkb):
                    exs[(6, kb)] = sc_exp(scT_ps, 0, 6, kb)
                    exs[(7, kb)] = sc_exp(scT_ps, 0, 7, kb)
                    if kb > 0:
                        pv_kb(pvs6, 6, kb - 1, exs[(6, kb - 1)], nkb)
                        pv_kb(pvs7, 7, kb - 1, exs[(7, kb - 1)], nkb)
                pv_kb(pvs6, 6, nkb - 1, exs[(6, nkb - 1)], nkb)
                pv_kb(pvs7, 7, nkb - 1, exs[(7, nkb - 1)], nkb)
                normalize(0, 6, pvs6)
                normalize(0, 7, pvs7)

            # ---- phase B: sl2=1 attention with the tt=0/1 out-projection
            #      matmuls spread through each group ----
            with ExitStack() as bctx:
                bpool = lambda *a, **k: bctx.enter_context(tc.tile_pool(*a, **k))
                scB_ps = bpool(name="scB_ps", bufs=2, space="PSUM")
                pvB_ps = bpool(name="pvB_ps", bufs=3, space="PSUM")
                po_ps = bpool(name="po_ps", bufs=1, space="PSUM")

                ochunks = [(tt, oc) for oc in range(4) for tt in range(2)]

                def o_finish(tt, oc, po):
                    o_sb = osb_p.tile([128, 512], F32, tag="osb")
                    nc.vector.tensor_copy(o_sb[:], po[:])
                    nc.sync.dma_start(
                        out[tt * 128:(tt + 1) * 128, 512 * oc:512 * (oc + 1)],
                        o_sb[:])

                nkb = CSLOT2[1]
                for g in range(8):
                    tt, oc = ochunks[g]
                    po = po_ps.tile([128, 512], F32, tag="po")
                    wo_bf = wo_bfs[oc]
                    pvs = [pvB_ps.tile([65, 4, 128], F32, tag="pv",
                                       name=f"pv{h}") for h in range(2)]
                    exs = {}
                    for kb in range(nkb):
                        exs[kb] = sc_exp(scB_ps, 1, g, kb)
                        for cht in (2 * kb, 2 * kb + 1):
                            nc.tensor.matmul(
                                po[:], attn_sb[:, cht, tt * 128:(tt + 1) * 128],
                                wo_bf[:, cht, :],
                                start=(cht == 0), stop=(cht == 15))
                        if kb > 0:
                            pv_kb(pvs, g, kb - 1, exs[kb - 1], nkb)
                    pv_kb(pvs, g, nkb - 1, exs[nkb - 1], nkb)
                    normalize(1, g, pvs)
                    o_finish(tt, oc, po)

                # remaining out-projection chunks (q tokens 256..511)
                for oc in range(4):
                    for tt in range(2, 4):
                        po = po_ps.tile([128, 512], F32, tag="po")
                        wo_bf = wo_bfs[oc]
                        for cht in range(16):
                            nc.tensor.matmul(
                                po[:], attn_sb[:, cht,
                                               tt * 128:(tt + 1) * 128],
                                wo_bf[:, cht, :],
                                start=(cht == 0), stop=(cht == 15))
                        o_finish(tt, oc, po)

    nc.finalize()
    return nc


def _core_rows(c):
    p, which = c // 2, c % 2
    if which == 0:
        rel = np.r_[np.arange(256), np.arange(768, 1024)]
        ctx = 1024
    else:
        rel = np.arange(256, 768)
        ctx = 768
    return p, rel, ctx


def _rot_host():
    rot = np.zeros((128, 128), np.float32)
    for o in (0, 64):
        for d in range(32):
            rot[o + 32 + d, o + d] = -1.0
            rot[o + d, o + 32 + d] = 1.0
    return rot.astype(ml_dtypes.bfloat16)


def _pack_hsT(hs):
    """[T, 2048] f32 -> [T//256, 128, 16, 256] bf16 (ki, ko, t)."""
    T = hs.shape[0]
    ht = hs.T.astype(ml_dtypes.bfloat16)            # [2048, T]
    ht = ht.reshape(16, 128, T // 256, 256)         # ko, ki, c, t
    return np.ascontiguousarray(ht.transpose(2, 1, 0, 3))


def _cs_table(x):
    """cos/sin [T, 32] f32 -> [128, T] bf16 broadcast over 4 row groups."""
    t = np.tile(x.T.astype(ml_dtypes.bfloat16), (4, 1))
    return np.ascontiguousarray(t)


_NC_CACHE = {}
_LAST_INMAPS = None


def kernel(hidden_states, cos, sin, Wq, Wk, Wv, Wo):
    hidden_states = np.ascontiguousarray(hidden_states, dtype=np.float32)
    cos = np.ascontiguousarray(cos, dtype=np.float32)
    sin = np.ascontiguousarray(sin, dtype=np.float32)
    Wq = np.ascontiguousarray(Wq, dtype=np.float32)
    Wk = np.ascontiguousarray(Wk, dtype=np.float32)
    Wv = np.ascontiguousarray(Wv, dtype=np.float32)
    Wo = np.ascontiguousarray(Wo, dtype=np.float32)

    if "nc" not in _NC_CACHE:
        _NC_CACHE["nc"] = build_nc()
    nc = _NC_CACHE["nc"]

    # weight packs (shared across cores)
    wq_s = np.ascontiguousarray(
        Wq.reshape(16, 128, 4, 2, 4, 64)            # ko ki a r i x
        .transpose(1, 2, 4, 0, 3, 5)                # ki a i ko r x
        .reshape(128, 16, 16, 128).astype(ml_dtypes.bfloat16))
    wk_s = np.ascontiguousarray(
        Wk.reshape(16, 128, 4, 128).transpose(1, 2, 0, 3)
        .astype(ml_dtypes.bfloat16))
    wv_s = np.ascontiguousarray(
        Wv.reshape(16, 128, 512).transpose(1, 0, 2).astype(ml_dtypes.bfloat16))
    wo_s = np.ascontiguousarray(
        Wo.reshape(16, 128, 4, 512).transpose(1, 2, 0, 3)
        .astype(ml_dtypes.bfloat16))
    rot = _rot_host()

    in_maps = []
    for c in range(8):
        p, rel, ctx = _core_rows(c)
        rows = p * S + rel
        hs_ctx = np.zeros((S, D), np.float32)
        hs_ctx[:ctx] = hidden_states[p * S:p * S + ctx]
        masks = np.ones((128, 2, 4, 256), np.float32)
        for sl2 in range(2):
            qabs = rel[sl2 * 256:(sl2 + 1) * 256]
            for mi, pos in enumerate(MASK_POS2[sl2]):
                kabs = pos * 128 + np.arange(128)
                masks[:, sl2, mi, :] = (qabs[None, :] >= kabs[:, None])
        in_maps.append(dict(
            hsct=_pack_hsT(hs_ctx),
            hsqt=_pack_hsT(np.ascontiguousarray(hidden_states[rows])),
            c4k=_cs_table(cos[p * S:(p + 1) * S]),
            s4k=_cs_table(sin[p * S:(p + 1) * S]),
            c4q=_cs_table(cos[p * S + rel]),
            s4q=_cs_table(sin[p * S + rel]),
            wqs=wq_s, wks=wk_s, wvs=wv_s, wos=wo_s,
            rot=rot, masks=masks.astype(ml_dtypes.bfloat16),
        ))

    global _LAST_INMAPS
    _LAST_INMAPS = in_maps

    last_err = None
    for _attempt in range(2):
        try:
            res = run_bass_kernel_spmd(nc, in_maps, core_ids=list(range(8)))
            break
        except Exception as e:  # one retry: device occasionally needs a reset
            last_err = e
    else:
        raise last_err

    outp = np.zeros((B * S, D), np.float32)
    for c in range(8):
        p, rel, ctx = _core_rows(c)
        outp[p * S + rel] = res.results[c]["out"]
    return outp


# revision 24
# speedup vs baseline: 1.0699x; 1.0318x over previous
"""Exaone GQA flash-attention block on 8 Trainium2 NeuronCores.

Sharding: each pair of cores (2p, 2p+1) handles prefill sequence p (S=1024).
Within a pair, q-tokens are split by 256-blocks {0,3} / {1,2} so causal attention
work balances; K/V are computed per-core for the context each core needs
(zero-padded to 1024). No cross-core communication: every core produces the
final output rows for its own 512 q-tokens; the host concatenates.

All data layout work (transposes, bf16 casts, weight slot packing, cos/sin
broadcast tables, causal masks) happens on the HOST; the device runs a pure
matmul/softmax pipeline:
  V/K projections stream over pre-transposed bf16 hsT; K gets NeoX rope via a
      +-1 rotation matmul and two multiply-adds; V lands naturally [tok, ch]
      with an appended ones column so the PV matmul also produces the softmax
      denominator
  Q projection per head-slot from pre-packed weight slots, roped into a
      [128, 16 slots, 512] layout (row halves = kv-head parity so score
      matmuls row-pack the PE array)
  scoresT = kT^T @ qT per (kv, 256-q-chunk, key-block), exp on ACT,
      multiplicative triangle masks on the possible diagonal positions
  attn^T accumulated in PSUM [65, 4 heads, 128], normalized by the broadcast
      reciprocal of the ones-row, written straight into the out-proj lhsT
      layout; out = attn^T.T @ Wo in 512-wide chunks, interleaved into the
      attention instruction stream to fill PE bubbles.
"""
import sys
sys.path.insert(0, '/opt/trn_rl_repo')

from contextlib import ExitStack

import ml_dtypes
import numpy as np

import concourse.bass as bass
import concourse.mybir as mybir
import concourse.tile as tile
from concourse import bacc
from concourse.bass_utils import run_bass_kernel_spmd

F32 = mybir.dt.float32
BF16 = mybir.dt.bfloat16
AF = mybir.ActivationFunctionType
MUL = mybir.AluOpType.mult
ADD = mybir.AluOpType.add

B, S, D = 4, 1024, 2048
HQ, HKV, HD = 32, 8, 64
SCALE = HD ** -0.5
NQ = 512                      # q tokens per core
CSLOT2 = (4, 8)               # key-blocks processed per 256-q-chunk (uniform)
MASK_POS2 = ((0, 1, 2, 3), (4, 5, 6, 7))  # masked kb positions per 256-chunk


def build_nc():
    nc = bacc.Bacc("TRN2", target_bir_lowering=False, debug=False,
                   num_devices=8, num_swdge_queues=4)

    # all inputs pre-cast/pre-laid-out by the host
    hsc_d = nc.dram_tensor("hsct", [4, 128, 16, 256], BF16, kind="ExternalInput")
    hsq_d = nc.dram_tensor("hsqt", [2, 128, 16, 256], BF16, kind="ExternalInput")
    c4k_d = nc.dram_tensor("c4k", [128, S], BF16, kind="ExternalInput")
    s4k_d = nc.dram_tensor("s4k", [128, S], BF16, kind="ExternalInput")
    c4q_d = nc.dram_tensor("c4q", [128, NQ], BF16, kind="ExternalInput")
    s4q_d = nc.dram_tensor("s4q", [128, NQ], BF16, kind="ExternalInput")
    wq_d = nc.dram_tensor("wqs", [128, 16, 16, 128], BF16, kind="ExternalInput")
    wk_d = nc.dram_tensor("wks", [128, 4, 16, 128], BF16, kind="ExternalInput")
    wv_d = nc.dram_tensor("wvs", [128, 16, 512], BF16, kind="ExternalInput")
    wo_d = nc.dram_tensor("wos", [128, 4, 16, 512], BF16, kind="ExternalInput")
    rot_in = nc.dram_tensor("rot", [128, 128], BF16, kind="ExternalInput")
    masks_in = nc.dram_tensor("masks", [128, 2, 4, 256], BF16, kind="ExternalInput")
    out = nc.dram_tensor("out", [NQ, D], F32, kind="ExternalOutput")

    with tile.TileContext(nc) as tc:
        with ExitStack() as ctx:
            pool = lambda *a, **k: ctx.enter_context(tc.tile_pool(*a, **k))
            qT_p = pool(name="qT", bufs=1)
            kT_p = pool(name="kT", bufs=1)
            v_p = pool(name="vsb", bufs=1)
            attn_p = pool(name="attn", bufs=1)
            const_p = pool(name="const", bufs=1)
            exp_p = pool(name="exps", bufs=6)
            rope_p = pool(name="rope", bufs=2)
            norm_p = pool(name="norm", bufs=1)
            osb_p = pool(name="osb", bufs=2)

            qT = qT_p.tile([128, 16, NQ], BF16)
            kT = kT_p.tile([128, 4, S], BF16)
            v_sb = v_p.tile([128, 8, 8, 65], BF16)
            attn_sb = attn_p.tile([128, 16, NQ], BF16)

            # ---- constants (tiles only; DMAs issued after the critical
            #      startup transfers) ----
            rot_bf = const_p.tile([128, 128], BF16)
            c4q = const_p.tile([128, NQ], BF16)
            s4q = const_p.tile([128, NQ], BF16)
            masks_bf = const_p.tile([128, 2, 4, 256], BF16)
            c4k = const_p.tile([128, S], BF16)
            s4k = const_p.tile([128, S], BF16)

            nc.vector.memset(v_sb[:, :, :, 64], 1.0)

            def rope(psum, rps_pool, c4, s4, col0, n, dst, cp_eng=None):
                """psum [128, n] -> dst (bf16) with NeoX rope applied."""
                x_sb = rope_p.tile([128, n], BF16, tag="rsb")
                (cp_eng or nc.scalar.copy)(x_sb[:], psum[:])
                pr = rps_pool.tile([128, n], F32, tag="rps")
                nc.tensor.matmul(pr[:], rot_bf[:], x_sb[:], start=True, stop=True)
                t1 = rope_p.tile([128, n], BF16, tag="rt1")
                nc.vector.tensor_tensor(t1[:], pr[:], s4[:, col0:col0 + n], MUL)
                t2 = rope_p.tile([128, n], BF16, tag="rt2")
                nc.vector.tensor_tensor(t2[:], x_sb[:], c4[:, col0:col0 + n], MUL)
                nc.vector.tensor_tensor(dst, t1[:], t2[:], ADD)

            # ---- shared attention helpers ----
            def sc_exp(sc_ps, sl2, g, kb):
                a, par = g // 2, g % 2
                base = 64 * par
                sc = sc_ps.tile([128, 2, 4, 128], F32, tag="sc")
                for h in range(2):
                    nc.tensor.matmul(
                        sc[:, h],
                        kT[base:base + 64, a, kb * 128:(kb + 1) * 128],
                        qT[base:base + 64, 4 * a:4 * a + 4,
                           (2 * sl2 + h) * 128:(2 * sl2 + h + 1) * 128],
                        start=True, stop=True)
                ex = exp_p.tile([128, 2, 4, 128], BF16, tag="ex")
                nc.scalar.activation(ex[:], sc[:], AF.Exp, scale=SCALE)
                if kb in MASK_POS2[sl2]:
                    mi = MASK_POS2[sl2].index(kb)
                    mk = masks_bf[:, sl2, mi].rearrange(
                        "p (two x) -> p two x", two=2)[:, :, None, :]
                    nc.vector.tensor_tensor(
                        ex[:], ex[:], mk.to_broadcast((128, 2, 4, 128)), MUL)
                return ex

            def pv_kb(pvs, g, kb, ex, nkb):
                for h in range(2):
                    nc.tensor.matmul(
                        pvs[h][:], v_sb[:, kb, g, :], ex[:, h],
                        start=(kb == 0), stop=(kb == nkb - 1))

            def normalize(sl2, g, pvs):
                l_sb = norm_p.tile([1, 2, 4, 128], F32, tag="lsb")
                nc.vector.tensor_copy(l_sb[:, 0], pvs[0][64:65, :, :])
                nc.vector.tensor_copy(l_sb[:, 1], pvs[1][64:65, :, :])
                rc2 = norm_p.tile([1, 2, 4, 128], F32, tag="recip")
                nc.vector.reciprocal_approx_fast(
                    rc2.rearrange("p a b q -> p (a b q)"),
                    l_sb.rearrange("p a b q -> p (a b q)"))
                rb2 = norm_p.tile([64, 2, 4, 128], F32, tag="rb")
                nc.gpsimd.partition_broadcast(rb2[:], rc2[:])
                for h in range(2):
                    sl = 2 * sl2 + h
                    pv = pvs[h]
                    pv_pair = pv[0:64].rearrange(
                        "p (i two) q -> p two i q", two=2)
                    rb_pair = rb2[:, h].rearrange(
                        "p (i two) q -> p two i q", two=2)
                    for par_o in range(2):
                        nc.vector.tensor_tensor(
                            attn_sb[64 * par_o:64 * par_o + 64,
                                    2 * g:2 * g + 2,
                                    sl * 128:(sl + 1) * 128],
                            pv_pair[:, par_o], rb_pair[:, par_o], MUL)

            # ---- phase A1: V + K projections ----
            with ExitStack() as ictx:
                ipool = lambda *a, **k: ictx.enter_context(tc.tile_pool(*a, **k))
                hsc_p = ipool(name="hsc", bufs=1)
                wk_p = ipool(name="wks", bufs=1)
                wv_p2 = ipool(name="wvs", bufs=1)
                proj_ps = ipool(name="proj_ps", bufs=3, space="PSUM")
                rot_ps = ipool(name="rot_ps", bufs=2, space="PSUM")

                # sync queue: wv half + hsc 2,3 + wk; scalar: hsc 0,1 then
                # the Q path; gpsimd: wv half, then k-rope tables + masks
                # startup-critical bytes first, in consumption order and
                # split so the first matmuls start after ~1.5MB; gpsimd
                # (slow SWDGE) carries nothing early
                wv_bf = wv_p2.tile([128, 16, 512], BF16)
                hs_ctxT = hsc_p.tile([128, 4, 16, 256], BF16)
                nc.sync.dma_start(wv_bf[:, 0:8], wv_d[:, 0:8])
                nc.scalar.dma_start(hs_ctxT[:, 0, 0:8], hsc_d[0, :, 0:8])
                nc.sync.dma_start(wv_bf[:, 8:16], wv_d[:, 8:16])
                nc.scalar.dma_start(hs_ctxT[:, 0, 8:16], hsc_d[0, :, 8:16])
                nc.scalar.dma_start(hs_ctxT[:, 1], hsc_d[1])
                wk_bf = wk_p.tile([128, 4, 16, 128], BF16)
                nc.scalar.dma_start(wk_bf[:], wk_d[:])
                for c in (2, 3):
                    nc.sync.dma_start(hs_ctxT[:, c], hsc_d[c])
                nc.scalar.dma_start(rot_bf[:], rot_in[:])
                nc.scalar.dma_start(c4k[:], c4k_d[:])
                nc.scalar.dma_start(s4k[:], s4k_d[:])
                nc.scalar.dma_start(c4q[:], c4q_d[:])
                nc.scalar.dma_start(s4q[:], s4q_d[:])
                nc.scalar.dma_start(masks_bf[:], masks_in[:])

                def v_tile(tt):
                    pv32 = proj_ps.tile([128, 512], F32, tag="proj")
                    for kt in range(16):
                        nc.tensor.matmul(
                            pv32[:],
                            hs_ctxT[:, tt // 2, kt,
                                    (tt % 2) * 128:(tt % 2) * 128 + 128],
                            wv_bf[:, kt, :], start=(kt == 0), stop=(kt == 15))
                    nc.vector.tensor_copy(
                        v_sb[:, tt, :, 0:64],
                        pv32.rearrange("p (g c) -> p g c", g=8))

                def k_chain(p, ch):
                    pk = proj_ps.tile([128, 512], F32, tag="proj")
                    for kt in range(16):
                        nc.tensor.matmul(
                            pk[:], wk_bf[:, p, kt, :],
                            hs_ctxT[:, 2 * ch:2 * ch + 2, kt, :],
                            start=(kt == 0), stop=(kt == 15))
                    rope(pk, rot_ps, c4k, s4k, 512 * ch, 512,
                         kT[:, p, 512 * ch:512 * (ch + 1)])

                v_tile(0)
                v_tile(1)
                v_tile(2)
                v_tile(3)
                k_chain(0, 0)
                k_chain(1, 0)
                k_chain(2, 0)
                k_chain(3, 0)
                v_tile(4)
                v_tile(5)
                v_tile(6)
                v_tile(7)
                k_chain(0, 1)
                k_chain(1, 1)
                k_chain(2, 1)
                k_chain(3, 1)

            # ---- phase A2: Q projection with the sl2=0 attention groups
            #      interleaved (Q matmuls keep the PE dense and warm while
            #      the exps run on the otherwise-idle ACT engine) ----
            wo_p = ctx.enter_context(tc.tile_pool(name="wo", bufs=4))
            wo_bfs = []
            for oc in range(4):
                wo_bf = wo_p.tile([128, 16, 512], BF16, tag="wobf")
                nc.sync.dma_start(wo_bf[:], wo_d[:, oc])
                wo_bfs.append(wo_bf)

            with ExitStack() as a2ctx:
                apool = lambda *a, **k: a2ctx.enter_context(tc.tile_pool(*a, **k))
                hsq_p = apool(name="hsq", bufs=1)
                wq_p = apool(name="wqs", bufs=3)
                proj2_ps = apool(name="proj2_ps", bufs=1, space="PSUM")
                rot2_ps = apool(name="rot2_ps", bufs=1, space="PSUM")
                scA_ps = apool(name="scA_ps", bufs=2, space="PSUM")
                pvA_ps = apool(name="pvA_ps", bufs=2, space="PSUM")

                hs_qT = hsq_p.tile([128, 2, 16, 256], BF16)
                for c in range(2):
                    nc.sync.dma_start(hs_qT[:, c], hsq_d[c])

                wq_tiles = {}

                def q_slot(s):
                    if s % 2 == 0:
                        wqa = wq_p.tile([128, 2, 16, 128], BF16, tag="wqa")
                        nc.scalar.dma_start(wqa[:], wq_d[:, s:s + 2])
                        wq_tiles[s // 2] = wqa
                    wqa = wq_tiles[s // 2]
                    pq = proj2_ps.tile([128, 512], F32, tag="proj")
                    for kt in range(16):
                        nc.tensor.matmul(
                            pq[:], wqa[:, s % 2, kt, :], hs_qT[:, :, kt, :],
                            start=(kt == 0), stop=(kt == 15))
                    cp = nc.scalar.copy if s % 2 == 0 else nc.vector.tensor_copy
                    rope(pq, rot2_ps, c4q, s4q, 0, NQ, qT[:, s, :], cp_eng=cp)

                def attA_group(g):
                    nkb = CSLOT2[0]
                    pvs = [pvA_ps.tile([65, 4, 128], F32, tag="pv",
                                       name=f"pv{h}") for h in range(2)]
                    exs = {}
                    for kb in range(nkb):
                        exs[kb] = sc_exp(scA_ps, 0, g, kb)
                        if kb > 0:
                            pv_kb(pvs, g, kb - 1, exs[kb - 1], nkb)
                    pv_kb(pvs, g, nkb - 1, exs[nkb - 1], nkb)
                    normalize(0, g, pvs)

                q_slot(0)
                q_slot(1)
                q_slot(2)
                q_slot(3)
                attA_group(0)
                q_slot(4)
                q_slot(5)
                attA_group(1)
                q_slot(6)
                q_slot(7)
                attA_group(2)
                q_slot(8)
                q_slot(9)
                attA_group(3)
                q_slot(10)
                q_slot(11)
                attA_group(4)
                q_slot(12)
                q_slot(13)
                attA_group(5)
                q_slot(14)
                q_slot(15)

            # ---- phase A2b: the last two sl2=0 groups have no Q filler
            #      left; run them as an interleaved pair (disjoint PE row
            #      groups) so each one's exp hides behind the other's MMs ----
            with ExitStack() as a2b:
                bpool2 = lambda *a, **k: a2b.enter_context(tc.tile_pool(*a, **k))
                scT_ps = bpool2(name="scT_ps", bufs=2, space="PSUM")
                pvT_ps = bpool2(name="pvT_ps", bufs=4, space="PSUM")
                nkb = CSLOT2[0]
                pvs6 = [pvT_ps.tile([65, 4, 128], F32, tag="pv",
                                    name=f"pv6{h}") for h in range(2)]
                pvs7 = [pvT_ps.tile([65, 4, 128], F32, tag="pv",
                                    name=f"pv7{h}") for h in range(2)]
                exs = {}
                for kb in range(nkb):
                    exs[(6, kb)] = sc_exp(scT_ps, 0, 6, kb)
                    exs[(7, kb)] = sc_exp(scT_ps, 0, 7, kb)
                    if kb > 0:
                        pv_kb(pvs6, 6, kb - 1, exs[(6, kb - 1)], nkb)
                        pv_kb(pvs7, 7, kb - 1, exs[(7, kb - 1)], nkb)
                pv_kb(pvs6, 6, nkb - 1, exs[(6, nkb - 1)], nkb)
                pv_kb(pvs7, 7, nkb - 1, exs[(7, nkb - 1)], nkb)
                normalize(0, 6, pvs6)
                normalize(0, 7, pvs7)

            # ---- phase B: sl2=1 attention with the tt=0/1 out-projection
            #      matmuls spread through each group ----
            with ExitStack() as bctx:
                bpool = lambda *a, **k: bctx.enter_context(tc.tile_pool(*a, **k))
                scB_ps = bpool(name="scB_ps", bufs=2, space="PSUM")
                pvB_ps = bpool(name="pvB_ps", bufs=3, space="PSUM")
                po_ps = bpool(name="po_ps", bufs=1, space="PSUM")

                ochunks = [(tt, oc) for oc in range(4) for tt in range(2)]

                def o_finish(tt, oc, po):
                    o_sb = osb_p.tile([128, 512], F32, tag="osb")
                    nc.vector.tensor_copy(o_sb[:], po[:])
                    nc.sync.dma_start(
                        out[tt * 128:(tt + 1) * 128, 512 * oc:512 * (oc + 1)],
                        o_sb[:])

                nkb = CSLOT2[1]
                for g in range(8):
                    tt, oc = ochunks[g]
                    po = po_ps.tile([128, 512], F32, tag="po")
                    wo_bf = wo_bfs[oc]
                    pvs = [pvB_ps.tile([65, 4, 128], F32, tag="pv",
                                       name=f"pv{h}") for h in range(2)]
                    exs = {}
                    for kb in range(nkb):
                        exs[kb] = sc_exp(scB_ps, 1, g, kb)
                        for cht in (2 * kb, 2 * kb + 1):
                            nc.tensor.matmul(
                                po[:], attn_sb[:, cht, tt * 128:(tt + 1) * 128],
                                wo_bf[:, cht, :],
                                start=(cht == 0), stop=(cht == 15))
                        if kb > 0:
                            pv_kb(pvs, g, kb - 1, exs[kb - 1], nkb)
                    pv_kb(pvs, g, nkb - 1, exs[nkb - 1], nkb)
                    normalize(1, g, pvs)
                    o_finish(tt, oc, po)

            # remaining out-projection chunks (q tokens 256..511) with a
            # double-buffered accumulator so chunk n+1 never waits on the
            # PSUM->SBUF copy of chunk n
            with ExitStack() as tctx:
                po2_ps = tctx.enter_context(
                    tc.tile_pool(name="po2_ps", bufs=2, space="PSUM"))
                for oc in range(4):
                    for tt in range(2, 4):
                        po = po2_ps.tile([128, 512], F32, tag="po")
                        wo_bf = wo_bfs[oc]
                        for cht in range(16):
                            nc.tensor.matmul(
                                po[:], attn_sb[:, cht,
                                               tt * 128:(tt + 1) * 128],
                                wo_bf[:, cht, :],
                                start=(cht == 0), stop=(cht == 15))
                        o_sb = osb_p.tile([128, 512], F32, tag="osb")
                        nc.vector.tensor_copy(o_sb[:], po[:])
                        nc.sync.dma_start(
                            out[tt * 128:(tt + 1) * 128,
                                512 * oc:512 * (oc + 1)], o_sb[:])

    nc.finalize()
    return nc


def _core_rows(c):
    p, which = c // 2, c % 2
    if which == 0:
        rel = np.r_[np.arange(256), np.arange(768, 1024)]
        ctx = 1024
    else:
        rel = np.arange(256, 768)
        ctx = 768
    return p, rel, ctx


def _rot_host():
    rot = np.zeros((128, 128), np.float32)
    for o in (0, 64):
        for d in range(32):
            rot[o + 32 + d, o + d] = -1.0
            rot[o + d, o + 32 + d] = 1.0
    return rot.astype(ml_dtypes.bfloat16)


def _pack_hsT(hs):
    """[T, 2048] f32 -> [T//256, 128, 16, 256] bf16 (ki, ko, t)."""
    T = hs.shape[0]
    ht = hs.T.astype(ml_dtypes.bfloat16)            # [2048, T]
    ht = ht.reshape(16, 128, T // 256, 256)         # ko, ki, c, t
    return np.ascontiguousarray(ht.transpose(2, 1, 0, 3))


def _cs_table(x):
    """cos/sin [T, 32] f32 -> [128, T] bf16 broadcast over 4 row groups."""
    t = np.tile(x.T.astype(ml_dtypes.bfloat16), (4, 1))
    return np.ascontiguousarray(t)


_NC_CACHE = {}
_LAST_INMAPS = None


def kernel(hidden_states, cos, sin, Wq, Wk, Wv, Wo):
    hidden_states = np.ascontiguousarray(hidden_states, dtype=np.float32)
    cos = np.ascontiguousarray(cos, dtype=np.float32)
    sin = np.ascontiguousarray(sin, dtype=np.float32)
    Wq = np.ascontiguousarray(Wq, dtype=np.float32)
    Wk = np.ascontiguousarray(Wk, dtype=np.float32)
    Wv = np.ascontiguousarray(Wv, dtype=np.float32)
    Wo = np.ascontiguousarray(Wo, dtype=np.float32)

    if "nc" not in _NC_CACHE:
        _NC_CACHE["nc"] = build_nc()
    nc = _NC_CACHE["nc"]

    # weight packs (shared across cores)
    wq_s = np.ascontiguousarray(
        Wq.reshape(16, 128, 4, 2, 4, 64)            # ko ki a r i x
        .transpose(1, 2, 4, 0, 3, 5)                # ki a i ko r x
        .reshape(128, 16, 16, 128).astype(ml_dtypes.bfloat16))
    wk_s = np.ascontiguousarray(
        Wk.reshape(16, 128, 4, 128).transpose(1, 2, 0, 3)
        .astype(ml_dtypes.bfloat16))
    wv_s = np.ascontiguousarray(
        Wv.reshape(16, 128, 512).transpose(1, 0, 2).astype(ml_dtypes.bfloat16))
    wo_s = np.ascontiguousarray(
        Wo.reshape(16, 128, 4, 512).transpose(1, 2, 0, 3)
        .astype(ml_dtypes.bfloat16))
    rot = _rot_host()

    in_maps = []
    for c in range(8):
        p, rel, ctx = _core_rows(c)
        rows = p * S + rel
        hs_ctx = np.zeros((S, D), np.float32)
        hs_ctx[:ctx] = hidden_states[p * S:p * S + ctx]
        masks = np.ones((128, 2, 4, 256), np.float32)
        for sl2 in range(2):
            qabs = rel[sl2 * 256:(sl2 + 1) * 256]
            for mi, pos in enumerate(MASK_POS2[sl2]):
                kabs = pos * 128 + np.arange(128)
                masks[:, sl2, mi, :] = (qabs[None, :] >= kabs[:, None])
        in_maps.append(dict(
            hsct=_pack_hsT(hs_ctx),
            hsqt=_pack_hsT(np.ascontiguousarray(hidden_states[rows])),
            c4k=_cs_table(cos[p * S:(p + 1) * S]),
            s4k=_cs_table(sin[p * S:(p + 1) * S]),
            c4q=_cs_table(cos[p * S + rel]),
            s4q=_cs_table(sin[p * S + rel]),
            wqs=wq_s, wks=wk_s, wvs=wv_s, wos=wo_s,
            rot=rot, masks=masks.astype(ml_dtypes.bfloat16),
        ))

    global _LAST_INMAPS
    _LAST_INMAPS = in_maps

    last_err = None
    for _attempt in range(2):
        try:
            res = run_bass_kernel_spmd(nc, in_maps, core_ids=list(range(8)))
            break
        except Exception as e:  # one retry: device occasionally needs a reset
            last_err = e
    else:
        raise last_err

    outp = np.zeros((B * S, D), np.float32)
    for c in range(8):
        p, rel, ctx = _core_rows(c)
        outp[p * S + rel] = res.results[c]["out"]
    return outp
